# revision 59
# baseline (speedup 1.0000x reference)
"""Trainium2 Bass kernel for AxialAttention (attention along W axis).

Sharding: pure data-parallel over (B=4) x (H split in 2) = 8 shards, one
per NeuronCore. Attention mixes pixels only along W within a single
(b, head, h-row), so splitting H requires no collectives.

The three K=512 projection GEMMs (qk / v / out) run as fp8e4m3
DoubleRow matmuls with 3-term error compensation:
    w @ x ~= wh @ (xh + xl) + wl @ xh
where (wh, wl) / (xh, xl) are hi/lo e4m3 pairs of power-of-2-scaled
operands (host-prepared for weights and x; on-device split for the
attention output). A DoubleRow matmul contracts two k-tiles packed
along the free dim at 0.5 cycles per output column, so pairing
(wh, wh)x(xh, xl) per 128-channel chunk plus (wl, wl)x(xh, xh) across
chunk pairs makes a K=512 3-term GEMM cost 0.75x bf16 at ~bf16
accuracy. The A-term weight duplicate is a stride-0 broadcast AP, not
stored. The attention itself (scores, exp, AV, sums) stays bf16:
with compensated operands fp8 DR saves nothing at K<=128, and any
uncompensated e4m3 operand fails the 2e-2 gate (measured).

Scales (power of 2, folded into activation scale / sums-ones value):
x*16, w*256 -> q,k,v carry 2^24 -> exp scale 0.125*2^-27-folded; the
sums-matmul ones are 128 so normalized attn out carries Sa=32;
out-proj psum = y*2^13 -> ACT Identity scale 2^-13 + bias; y stored
bf16 and upcast on host.

Schedule: per 384-pixel group, qk(t+1) block-pairs interleave with
rows(t); row backs lag fronts by depth=1; q/k evacuate via one ACT
copy per [128,1024] psum pair-tile; the a-pair chain is DVE mult ->
Pool quantize -> Pool residual; the last two out-projections defer
into the epilogue so the final a-chain overlaps their PE work.
"""

import numpy as np
import ml_dtypes

import concourse.bass as bass
import concourse.tile as tile
from concourse import mybir

BF16 = mybir.dt.bfloat16
F32 = mybir.dt.float32
E4 = mybir.dt.float8e4
E4NP = ml_dtypes.float8_e4m3
DR = mybir.MatmulPerfMode.DoubleRow

B, C, H, W = 4, 512, 96, 96
HEADS, D = 8, 64
NCORES = 8
RPC = H // 2          # 48 rows per core
PIX = RPC * W         # 4608 pixels per core
GRP = 12              # pixel groups
GPIX = PIX // GRP     # pixels per group
RPG = GPIX // 96      # attention rows per group
NP = max(1, GPIX // 192)  # 192-col N-pieces per DR matmul

SX, SW, SA = 16.0, 256.0, 32.0
EXP_SCALE = 0.125 / (SX * SW) ** 2          # 2^-27
OUT_SCALE = 1.0 / (SA * SW)                 # 2^-13
ONES_VAL = SX * SW / SA                     # 128.0


DEFAULT_CFG = dict(
    warmup=8,           # dummy PE matmuls covering the input-DMA window
    psa_bufs=2,         # PSUM [128,512] pool depth (v / out projs)
    sums_first=False,   # emit sums matmuls before AV so recip overlaps
    ah_eng='pool',      # engine for the a-hi quantize copy: dve|act|pool
    al_eng='pool',      # engine for the a-lo residual subtract: dve|pool
    qk_evac='act',      # act | split (alternate act/dve per block)
    outproj_at=4,       # rr at which outproj(t-1) is emitted (4 = after)
    eager_last=True,    # depth-1 drain in the last group
    dma_v2=False,       # split wqk DMA for earlier qk start
    depth=1,            # row-back pipeline lag
    qk_pair=True,       # qk psum in [128,1024] psB tiles, paired evac
    psb_bufs=3,         # PSUM [128,1024] pool depth
    dma_one_q=False,    # all input DMAs on the Pool queue, strict order
    last_fast=True,     # all-DVE a-pair chain in the last group (tail)
    dma_v3=False,       # critical loads (x0,wqkA,wqkB,wv,x1) on SP queue
    out_dve=False,      # out-proj evac on DVE via scalar_tensor_tensor
    y_bf16=True,        # emit y in bf16, upcast on host
    tail_defer=2,       # outprojs deferred into the epilogue (1 or 2)
    qkpool_bufs=2,
    qk_interleave=True,   # spread qk(t+1) pairs across rows(t)
    dma_v4=False,       # split startup loads on SP queue, dep order
    tail_multiq=False,  # spread deferred-group y stores across queues
    exp_split=False,    # exp in two per-parity ACT ops
    ah_direct=False,    # normalize-mult writes a-hi e4m3 directly
    apool_bufs=3,
    vrow_bufs=6,
    attn_bufs=6,
    ostage_bufs=4,
)


def build_nc(apply_waitfix=True, cfg=None):
    kcfg = dict(DEFAULT_CFG)
    if cfg:
        kcfg.update(cfg)
    nc = bass.Bass(trn_type="TRN2")
    x_d = nc.declare_dram_parameter("x", [128, GRP * 8 * GPIX // 1], E4, isOutput=False)
    wqk_d = nc.declare_dram_parameter("wqk", [128, 8192], E4, isOutput=False)
    wv_d = nc.declare_dram_parameter("wv", [128, 4096], E4, isOutput=False)
    wo_d = nc.declare_dram_parameter("wo", [128, 4096], E4, isOutput=False)
    bias_d = nc.declare_dram_parameter("bias", [4, 128, 1], F32, isOutput=False)
    y_d = nc.declare_dram_parameter(
        "y", [512, PIX], BF16 if kcfg['y_bf16'] else F32, isOutput=True)

    with tile.TileContext(nc) as tc:
        with (
            tc.tile_pool(name="persist", bufs=1) as persist,
            tc.tile_pool(name="qkpool", bufs=kcfg['qkpool_bufs']) as qkpool,
            tc.tile_pool(name="vrow", bufs=kcfg['vrow_bufs']) as vrow,
            tc.tile_pool(name="attn", bufs=kcfg['attn_bufs']) as attn,
            tc.tile_pool(name="apool", bufs=kcfg['apool_bufs']) as apool,
            tc.tile_pool(name="ostage", bufs=kcfg['ostage_bufs']) as ostage,
            tc.tile_pool(name="psA", bufs=kcfg['psa_bufs'],
                         space="PSUM") as psA,
            tc.tile_pool(name="psB", bufs=kcfg['psb_bufs'],
                         space="PSUM") as psB,
        ):
            # --- PE warmup: dependency-free dummy matmuls fill the
            # initial DMA wait and finish the clock ramp ---------------
            warm_sb = persist.tile([128, 512], BF16, tag="warm")
            nc.vector.memset(warm_sb[:, :], 0.0)
            wps = psA.tile([128, 512], F32, tag="psA")
            for _ in range(kcfg['warmup']):
                nc.tensor.matmul(wps[:, 0:512], lhsT=warm_sb[:, 0:128],
                                 rhs=warm_sb[:, :])

            # --- persistent loads (first x group + q/k weights first so
            # the projection GEMMs start early; wqkA split so blocks 0-1
            # can start before the whole tensor lands) -----------------
            x_t = [None] * GRP
            wqk_t = persist.tile([128, 8192], E4, tag="wqk")
            wqkA_t = wqk_t[:, 0:4096]
            wqkB_t = wqk_t[:, 4096:8192]
            if kcfg['dma_v4']:
                # dependency-ordered pieces, all on the SP/HWDGE queue:
                # x0, wqk-q(A,B), wqk-k(A,B), wv, x1; rest via gpsimd
                xt0 = persist.tile([128, 8 * GPIX], E4, tag="x_c0")
                nc.sync.dma_start(out=xt0[:, :], in_=x_d[:, 0:8 * GPIX])
                x_t[0] = xt0[:, :]
                nc.sync.dma_start(out=wqk_t[:, 0:2048],
                                  in_=wqk_d[:, 0:2048])       # A q
                nc.sync.dma_start(out=wqk_t[:, 4096:6144],
                                  in_=wqk_d[:, 4096:6144])    # B q
                nc.sync.dma_start(out=wqk_t[:, 2048:4096],
                                  in_=wqk_d[:, 2048:4096])    # A k
                nc.sync.dma_start(out=wqk_t[:, 6144:8192],
                                  in_=wqk_d[:, 6144:8192])    # B k
                wv_t = persist.tile([128, 4096], E4, tag="wv")
                nc.sync.dma_start(out=wv_t[:, :], in_=wv_d[:, :])
                ones_t = persist.tile([96, 64], BF16, tag="ones")
                nc.vector.memset(ones_t[:, :], ONES_VAL)
                xt1 = persist.tile([128, 8 * GPIX], E4, tag="x_c1")
                nc.sync.dma_start(out=xt1[:, :],
                                  in_=x_d[:, 8 * GPIX:16 * GPIX])
                x_t[1] = xt1[:, :]
                wo_t = persist.tile([128, 4096], E4, tag="wo")
                nc.gpsimd.dma_start(out=wo_t[:, :], in_=wo_d[:, :])
                bias_t = []
                for cc in range(4):
                    bt = persist.tile([128, 1], F32, tag=f"bias{cc}")
                    nc.gpsimd.dma_start(out=bt[:, :], in_=bias_d[cc])
                    bias_t.append(bt)
                for ci, (t0, t1) in enumerate([(2, 4), (4, 6), (6, 9),
                                               (9, 12)]):
                    GB = 8 * GPIX
                    xt = persist.tile([128, (t1 - t0) * GB], E4,
                                      tag=f"x_c{ci + 2}")
                    nc.sync.dma_start(out=xt[:, :],
                                      in_=x_d[:, t0 * GB:t1 * GB])
                    for t in range(t0, t1):
                        x_t[t] = xt[:, (t - t0) * GB:(t - t0 + 1) * GB]
            elif kcfg['dma_v2']:
                # q-halves first so qk(0) blocks 0-3 can start earliest
                nc.gpsimd.dma_start(out=wqk_t[:, 0:2048],
                                    in_=wqk_d[:, 0:2048])      # A q
                nc.gpsimd.dma_start(out=wqk_t[:, 4096:6144],
                                    in_=wqk_d[:, 4096:6144])   # B q
                nc.gpsimd.dma_start(out=wqk_t[:, 2048:4096],
                                    in_=wqk_d[:, 2048:4096])   # A k
                nc.gpsimd.dma_start(out=wqk_t[:, 6144:8192],
                                    in_=wqk_d[:, 6144:8192])   # B k
            if kcfg['dma_v4']:
                pass
            elif kcfg['dma_v3']:
                # dependency-ordered critical loads on the fast SP queue
                xt0 = persist.tile([128, 8 * GPIX], E4, tag="x_c0")
                nc.sync.dma_start(out=xt0[:, :], in_=x_d[:, 0:8 * GPIX])
                x_t[0] = xt0[:, :]
                nc.sync.dma_start(out=wqk_t[:, 0:4096],
                                  in_=wqk_d[:, 0:4096])
                nc.sync.dma_start(out=wqk_t[:, 4096:],
                                  in_=wqk_d[:, 4096:])
                wv_t = persist.tile([128, 4096], E4, tag="wv")
                nc.sync.dma_start(out=wv_t[:, :], in_=wv_d[:, :])
                ones_t = persist.tile([96, 64], BF16, tag="ones")
                nc.vector.memset(ones_t[:, :], ONES_VAL)
                if kcfg['out_dve']:
                    c13_t = persist.tile([128, GPIX], F32, tag="c13")
                    nc.vector.memset(c13_t[:, :], OUT_SCALE)
                xt1 = persist.tile([128, 8 * GPIX], E4, tag="x_c1")
                nc.sync.dma_start(out=xt1[:, :],
                                  in_=x_d[:, 8 * GPIX:16 * GPIX])
                x_t[1] = xt1[:, :]
                wo_t = persist.tile([128, 4096], E4, tag="wo")
                nc.gpsimd.dma_start(out=wo_t[:, :], in_=wo_d[:, :])
                bias_t = []
                bias13_t = []
                for cc in range(4):
                    bt = persist.tile([128, 1], F32, tag=f"bias{cc}")
                    nc.gpsimd.dma_start(out=bt[:, :], in_=bias_d[cc])
                    bias_t.append(bt)
                    if kcfg['out_dve']:
                        b13 = persist.tile([128, 1], F32, tag=f"b13_{cc}")
                        nc.vector.tensor_scalar_mul(
                            out=b13[:, :], in0=bt[:, :],
                            scalar1=1.0 / OUT_SCALE)
                        bias13_t.append(b13)
                CHUNKS = [(2, 4), (4, 6), (6, 9), (9, 12)]
                for ci, (t0, t1) in enumerate(CHUNKS):
                    xt = persist.tile([128, (t1 - t0) * 3072], E4,
                                      tag=f"x_c{ci + 2}")
                    nc.sync.dma_start(out=xt[:, :],
                                      in_=x_d[:, t0 * 3072:t1 * 3072])
                    for t in range(t0, t1):
                        x_t[t] = xt[:, (t - t0) * 3072:
                                    (t - t0 + 1) * 3072]
            else:
                if not kcfg['dma_v2']:
                    nc.gpsimd.dma_start(out=wqk_t[:, :], in_=wqk_d[:, :])
                xq = nc.gpsimd if kcfg['dma_one_q'] else nc.sync
                GB = 8 * GPIX
                if GRP == 12:
                    CHUNKS = [(0, 1), (1, 2), (2, 4), (4, 6), (6, 9),
                              (9, 12)]
                else:
                    CHUNKS = [(0, 1), (1, 2), (2, 4), (4, 8), (8, 16),
                              (16, 24)]
                for ci, (t0, t1) in enumerate(CHUNKS):
                    xt = persist.tile([128, (t1 - t0) * GB], E4,
                                      tag=f"x_c{ci}")
                    xq.dma_start(out=xt[:, :],
                                 in_=x_d[:, t0 * GB:t1 * GB])
                    for t in range(t0, t1):
                        x_t[t] = xt[:, (t - t0) * GB:(t - t0 + 1) * GB]
                    if ci == 0:
                        wv_t = persist.tile([128, 4096], E4, tag="wv")
                        nc.gpsimd.dma_start(out=wv_t[:, :], in_=wv_d[:, :])
                        ones_t = persist.tile([96, 64], BF16, tag="ones")
                        nc.vector.memset(ones_t[:, :], ONES_VAL)
                        if kcfg['out_dve']:
                            c13_t = persist.tile([128, GPIX], F32,
                                                 tag="c13")
                            nc.vector.memset(c13_t[:, :], OUT_SCALE)
                    elif ci == 1:
                        wo_t = persist.tile([128, 4096], E4, tag="wo")
                        nc.gpsimd.dma_start(out=wo_t[:, :], in_=wo_d[:, :])
                        bias_t = []
                        bias13_t = []
                        for cc in range(4):
                            bt = persist.tile([128, 1], F32,
                                              tag=f"bias{cc}")
                            nc.sync.dma_start(out=bt[:, :], in_=bias_d[cc])
                            bias_t.append(bt)
                            if kcfg['out_dve']:
                                b13 = persist.tile([128, 1], F32,
                                                   tag=f"b13_{cc}")
                                nc.vector.tensor_scalar_mul(
                                    out=b13[:, :], in0=bt[:, :],
                                    scalar1=1.0 / OUT_SCALE)
                                bias13_t.append(b13)
            wvA_t = wv_t[:, 0:2048]
            wvB_t = wv_t[:, 2048:4096]
            woA_t = wo_t[:, 0:2048]
            woB_t = wo_t[:, 2048:4096]

            one_f32 = persist.tile([128, 384], F32, tag="one_f32")
            nc.vector.memset(one_f32[:, :], 1.0)

            qk_t = {}             # group t -> list of 8 bf16 q/k tiles
            a_t = [None] * GRP

            def emit_qk(t):
                """q/k projection for group t: fp8 DR 3-term GEMM."""
                xA = x_t[t].rearrange("p (c hl n) -> p c hl n", c=4, hl=2)
                xB = x_t[t].rearrange("p (pr c2 hl n) -> p pr hl c2 n",
                                      pr=2, c2=2, hl=2)
                lA = wqkA_t.rearrange("p (o c m) -> p o c m", o=8, c=4)
                lB = wqkB_t.rearrange("p (o pr two m) -> p o pr two m",
                                      o=8, pr=2, two=2)
                qk_t[t] = []

                def qk_block(oc, qps, base):
                    for nn in range(NP):
                        nsl = slice(base + nn * 192, base + nn * 192 + 192)
                        xsl = slice(nn * 192, nn * 192 + 192)
                        for cc in range(4):
                            nc.tensor.matmul(
                                qps[:, nsl],
                                lhsT=lA[:, oc, cc].unsqueeze(1)
                                .broadcast_to([128, 2, 128]),
                                rhs=xA[:, cc, :, xsl],
                                start=(cc == 0), stop=False, perf_mode=DR)
                        for pp in range(2):
                            nc.tensor.matmul(
                                qps[:, nsl], lhsT=lB[:, oc, pp],
                                rhs=xB[:, pp, 0, :, xsl],
                                start=False, stop=(pp == 1), perf_mode=DR)

                if kcfg['qk_pair']:
                    def emit_pair(op_):
                        qps = psB.tile([128, 1024], F32, tag="psB",
                                       name="qkps")
                        qk_block(2 * op_, qps, 0)
                        qk_block(2 * op_ + 1, qps, 512)
                        qt = qkpool.tile([128, 2 * GPIX], BF16,
                                         tag=f"qkp{op_}", name="qkp")
                        nc.scalar.copy(
                            out=qt.rearrange("p (k m) -> p k m", k=2),
                            in_=qps[:, :].rearrange("p (k m) -> p k m",
                                                    k=2)[:, :, 0:GPIX])
                        qk_t[t].append(qt[:, 0:GPIX])
                        qk_t[t].append(qt[:, GPIX:2 * GPIX])
                    if kcfg['qk_interleave']:
                        return [lambda op_=op_: emit_pair(op_)
                                for op_ in range(4)]
                    for op_ in range(4):
                        emit_pair(op_)
                else:
                    for oc in range(8):
                        qps = psA.tile([128, 512], F32, tag="psA")
                        qk_block(oc, qps, 0)
                        qt = qkpool.tile([128, GPIX], BF16, tag=f"qk{oc}")
                        if kcfg['qk_evac'] == 'act' or oc % 2 == 0:
                            nc.scalar.copy(out=qt[:, :], in_=qps[:, 0:GPIX])
                        else:
                            nc.vector.tensor_copy(out=qt[:, :],
                                                  in_=qps[:, 0:GPIX])
                        qk_t[t].append(qt)

            def emit_row_front(t, rr):
                """v projection + scores + exp for row rr of group t."""
                xvA = x_t[t].rearrange("p (c hl r j) -> p c hl r j",
                                       c=4, hl=2, r=RPG)
                xvB = x_t[t].rearrange("p (pr c2 hl r j) -> p pr hl c2 r j",
                                       pr=2, c2=2, hl=2, r=RPG)
                vA = wvA_t.rearrange("p (c m) -> p c m", c=4)
                vB = wvB_t.rearrange("p (pr two m) -> p pr two m", pr=2, two=2)
                vps = psA.tile([128, 512], F32, tag="psA")
                for nn in range(2):
                    nsl = slice(nn * 256, nn * 256 + 256)
                    for cc in range(4):
                        nc.tensor.matmul(
                            vps[0:96, nsl], lhsT=xvA[:, cc, :, rr],
                            rhs=vA[:, cc, nsl].unsqueeze(1)
                            .broadcast_to([128, 2, 256]),
                            start=(cc == 0), stop=False, perf_mode=DR)
                    for pp in range(2):
                        nc.tensor.matmul(
                            vps[0:96, nsl], lhsT=xvB[:, pp, 0, :, rr],
                            rhs=vB[:, pp, :, nsl],
                            start=False, stop=(pp == 1), perf_mode=DR)
                v_sb = vrow.tile([96, 512], BF16)
                nc.vector.tensor_copy(out=v_sb[:, :], in_=vps[0:96, 0:512])

                # scores^T per head: [j, i]; head h -> bank h%2
                rsl = slice(rr * 96, rr * 96 + 96)
                sps = psB.tile([128, 1024], F32, tag="psB")
                for h in range(8):
                    qc, half = h // 2, 64 * (h % 2)
                    col = 512 * (h % 2) + 96 * (h // 2)
                    nc.tensor.matmul(
                        sps[0:96, col:col + 96],
                        lhsT=qk_t[t][4 + qc][half:half + 64, rsl],
                        rhs=qk_t[t][qc][half:half + 64, rsl],
                    )
                expS = attn.tile([96, 768], BF16)
                if kcfg['exp_split']:
                    for par in range(2):
                        nc.scalar.activation(
                            out=expS[:, par * 384:(par + 1) * 384],
                            in_=sps[0:96, par * 512:par * 512 + 384],
                            func=mybir.ActivationFunctionType.Exp,
                            scale=EXP_SCALE,
                        )
                else:
                    nc.scalar.activation(
                        out=expS.rearrange("p (k n) -> p k n", k=2),
                        in_=sps[0:96, :].rearrange("p (k m) -> p k m", k=2)
                            [:, :, 0:384],
                        func=mybir.ActivationFunctionType.Exp,
                        scale=EXP_SCALE,
                    )
                return v_sb, expS

            def emit_row_back(t, rr, v_sb, expS):
                """AV + sums matmuls, recip, normalize, a-pair split.

                Sums go first so the DVE reciprocal overlaps the AV
                matmuls; the whole a-pair chain (mult, quantize,
                residual) runs back-to-back on DVE to minimize the
                latency seen by the downstream out-projection.
                """
                aps = psB.tile([128, 1024], F32, tag="psB")

                def emit_sums():
                    for par in range(2):
                        nc.tensor.matmul(
                            aps[64 * par:64 * par + 64, 512:896],
                            lhsT=ones_t[:, :],
                            rhs=expS[:, 384 * par:384 * par + 384],
                        )

                recip = attn.tile([128, 384], F32)
                if kcfg['sums_first']:
                    emit_sums()
                    nc.vector.reciprocal(out=recip[:, :],
                                         in_=aps[:, 512:896])
                for h in range(8):
                    half, blk = 64 * (h % 2), 96 * (h // 2)
                    ecol = 384 * (h % 2) + 96 * (h // 2)
                    nc.tensor.matmul(
                        aps[half:half + 64, blk:blk + 96],
                        lhsT=v_sb[:, h * 64:(h + 1) * 64],
                        rhs=expS[:, ecol:ecol + 96],
                    )
                if not kcfg['sums_first']:
                    emit_sums()
                    nc.vector.reciprocal(out=recip[:, :],
                                         in_=aps[:, 512:896])
                asc = apool.tile([128, 384], F32, tag="asc")
                ar = a_t[t].rearrange("p (c hl r j) -> p c hl r j",
                                      c=4, hl=2, r=RPG)
                asr = asc.rearrange("p (c j) -> p c j", c=4)
                fast = kcfg['last_fast'] and t == GRP - 1 and rr >= 2
                in0c = aps[:, 0:384].rearrange("p (c i) -> p c i", c=4)
                in1c = recip.rearrange("p (c i) -> p c i", c=4)
                if kcfg['ah_direct']:
                    # a-hi straight out of the normalize multiply, then
                    # the f32 copy for the residual source
                    nc.vector.tensor_tensor(
                        out=ar[:, :, 0, rr], in0=in0c, in1=in1c,
                        op=mybir.AluOpType.mult)
                    nc.vector.tensor_tensor(
                        out=asr, in0=in0c, in1=in1c,
                        op=mybir.AluOpType.mult)
                else:
                    nc.vector.tensor_tensor(out=asr, in0=in0c, in1=in1c,
                                            op=mybir.AluOpType.mult)
                    if kcfg['ah_eng'] == 'dve' or fast:
                        nc.vector.tensor_copy(out=ar[:, :, 0, rr],
                                              in_=asr)
                    elif kcfg['ah_eng'] == 'pool':
                        nc.gpsimd.tensor_tensor(
                            out=ar[:, :, 0, rr], in0=asr,
                            in1=one_f32[:, :]
                            .rearrange("p (c j) -> p c j", c=4),
                            op=mybir.AluOpType.mult)
                    else:
                        nc.scalar.copy(out=ar[:, :, 0, rr], in_=asr)
                al_eng = (nc.vector if kcfg['al_eng'] == 'dve' or fast
                          else nc.gpsimd)
                al_eng.tensor_tensor(
                    out=ar[:, :, 1, rr], in0=asr, in1=ar[:, :, 0, rr],
                    op=mybir.AluOpType.subtract,
                )

            def emit_outproj(t, yq=None):
                aA = a_t[t].rearrange("p (c hl n) -> p c hl n", c=4, hl=2)
                aB = a_t[t].rearrange("p (pr c2 hl n) -> p pr hl c2 n",
                                      pr=2, c2=2, hl=2)
                lA = woA_t.rearrange("p (o c m) -> p o c m", o=4, c=4)
                lB = woB_t.rearrange("p (o pr two m) -> p o pr two m",
                                     o=4, pr=2, two=2)
                ydt = BF16 if kcfg['y_bf16'] else F32
                for op_ in range(2):
                    o_sb = ostage.tile([128, 2 * GPIX], ydt, name="o_sb")
                    for sub in range(2):
                        oc = 2 * op_ + sub
                        ops_ = psA.tile([128, 512], F32, tag="psA")
                        for nn in range(NP):
                            nsl = slice(nn * 192, nn * 192 + 192)
                            for cc in range(4):
                                nc.tensor.matmul(
                                    ops_[:, nsl],
                                    lhsT=lA[:, oc, cc].unsqueeze(1)
                                    .broadcast_to([128, 2, 128]),
                                    rhs=aA[:, cc, :, nsl],
                                    start=(cc == 0), stop=False,
                                    perf_mode=DR)
                            for pp in range(2):
                                nc.tensor.matmul(
                                    ops_[:, nsl], lhsT=lB[:, oc, pp],
                                    rhs=aB[:, pp, 0, :, nsl],
                                    start=False, stop=(pp == 1),
                                    perf_mode=DR)
                        osl = o_sb[:, sub * GPIX:(sub + 1) * GPIX]
                        if kcfg['out_dve']:
                            # (psum + bias*2^13) * 2^-13
                            nc.vector.scalar_tensor_tensor(
                                out=osl, in0=ops_[:, 0:GPIX],
                                scalar=bias13_t[oc][:, :], in1=c13_t[:, :],
                                op0=mybir.AluOpType.add,
                                op1=mybir.AluOpType.mult)
                        else:
                            nc.scalar.activation(
                                out=osl, in_=ops_[:, 0:GPIX],
                                func=mybir.ActivationFunctionType.Identity,
                                scale=OUT_SCALE, bias=bias_t[oc][:, :])
                    q = nc.sync if yq is None else yq[op_]
                    q.dma_start(
                        out=y_d[op_ * 256:(op_ + 1) * 256,
                                t * GPIX:(t + 1) * GPIX]
                        .rearrange("(k p) n -> p k n", k=2),
                        in_=o_sb.rearrange("p (k n) -> p k n", k=2))

            # software pipeline: qk(t+1) ahead of rows(t); AV stage one
            # row behind scores so PE always has independent work while
            # ACT computes exp / DVE evacuates
            for _p in (emit_qk(0) or []):
                _p()
            from collections import deque
            pend = deque()
            DEPTH = kcfg['depth']
            for t in range(GRP):
                qk_parts = []
                if t + 1 < GRP:
                    qk_parts = emit_qk(t + 1) or []
                a_t[t] = apool.tile([128, 8 * GPIX], E4, tag="ahat",
                                    name="ahat")
                ppr = (len(qk_parts) + RPG - 1) // RPG if qk_parts else 0
                for rr in range(RPG):
                    for k in range(ppr * rr, min(ppr * (rr + 1),
                                                 len(qk_parts))):
                        qk_parts[k]()
                    front = emit_row_front(t, rr)
                    if len(pend) >= DEPTH:
                        emit_row_back(*pend.popleft())
                    # drain eagerly in the last group to shorten the tail
                    if (kcfg['eager_last'] and t == GRP - 1 and rr >= 1
                            and pend):
                        emit_row_back(*pend.popleft())
                    pend.append((t, rr) + front)
                    if rr == kcfg['outproj_at'] and t >= 1:
                        emit_outproj(t - 1)
                if (kcfg['outproj_at'] == 4 and t >= 1
                        and t < GRP - kcfg['tail_defer']):
                    emit_outproj(t - 1)
            # epilogue: the last a-pair chain overlaps deferred outprojs
            while pend:
                emit_row_back(*pend.popleft())
            QS = ([[nc.scalar, nc.sync], [nc.sync, nc.scalar],
                   [nc.scalar, nc.sync]] if kcfg['tail_multiq']
                  else [None, None, None])
            qi = 0
            if kcfg['outproj_at'] == 4:
                for tt in range(GRP - kcfg['tail_defer'], GRP):
                    emit_outproj(tt - 1, yq=QS[qi])
                    qi += 1
            emit_outproj(GRP - 1, yq=QS[qi])

    if apply_waitfix:
        split_excess_waits(nc)
    return nc


# --- walrus workaround -------------------------------------------------
# The walrus build in this container rejects instructions carrying more
# than a small number of semaphore waits (1 for CTRL-queue NoOp/Drain).
# TileContext's exit drain can exceed that. Split: keep at most one wait
# on the original instruction and insert same-engine NoOps immediately
# before it, each carrying one of the excess waits.
def split_excess_waits(nc):
    import bass_rust
    n_split = 0
    for f in nc.m.functions:
        for blk in f.blocks:
            newlist = []
            changed = False
            for inst in blk.instructions:
                si = inst.sync_info
                w = list(si.on_wait) if si is not None else []
                if len(w) > 1:
                    *pre, last = w
                    for ci, wait in enumerate(pre):
                        nop = mybir.InstNoOp(
                            name=f"{inst.name}-wsplit{ci}", ins=[], outs=[])
                        nop.engine = inst.engine
                        nop.sync_info = bass_rust.SyncInfo(
                            on_update=[], on_wait=[wait])
                        newlist.append(nop)
                    inst.sync_info.on_wait = [last]
                    changed = True
                    n_split += 1
                newlist.append(inst)
            if changed:
                blk.instructions = newlist
    return n_split


def _pair(a, s):
    """Power-of-2 prescale + e4m3 hi/lo split (f32 in, e4m3 out x2)."""
    a = a.astype(np.float32) * s
    hi = a.astype(E4NP)
    lo = (a - hi.astype(np.float32)).astype(E4NP)
    return hi, lo


def shard_inputs(x, w_qkv, w_out, b_out):
    """Full inputs -> list of 8 per-core input maps."""
    x = np.asarray(x, dtype=np.float32)
    w_qkv = np.asarray(w_qkv, dtype=np.float32)
    w_out = np.asarray(w_out, dtype=np.float32)
    b_out = np.asarray(b_out, dtype=np.float32)

    # q/k projection weights as lhsT [in 512, out 1024], fp8 pair
    wh, wl = _pair(np.ascontiguousarray(w_qkv[:1024].T), SW)
    whr = wh.reshape(4, 128, 8, 128)           # (c, p, o, m)
    wlr = wl.reshape(4, 128, 8, 128)
    wqkA = np.ascontiguousarray(
        whr.transpose(1, 2, 0, 3)).reshape(128, 4096)
    wqkB = np.ascontiguousarray(
        wlr.reshape(2, 2, 128, 8, 128).transpose(2, 3, 0, 1, 4)
    ).reshape(128, 4096)

    # v projection weights as rhs [in 512, out 512], fp8 pair
    vh, vl = _pair(np.ascontiguousarray(w_qkv[1024:].T), SW)
    vhr = vh.reshape(4, 128, 512)
    wvA = np.ascontiguousarray(vhr.transpose(1, 0, 2)).reshape(128, 2048)
    wvB = np.ascontiguousarray(
        vl.reshape(2, 2, 128, 512).transpose(2, 0, 1, 3)).reshape(128, 2048)

    # out projection weights as lhsT [in 512, out 512], fp8 pair
    oh, ol = _pair(np.ascontiguousarray(w_out.T), SW)
    ohr = oh.reshape(4, 128, 4, 128)
    olr = ol.reshape(4, 128, 4, 128)
    woA = np.ascontiguousarray(
        ohr.transpose(1, 2, 0, 3)).reshape(128, 2048)
    woB = np.ascontiguousarray(
        olr.reshape(2, 2, 128, 4, 128).transpose(2, 3, 0, 1, 4)
    ).reshape(128, 2048)

    wqk = np.concatenate([wqkA, wqkB], axis=1)
    wv = np.concatenate([wvA, wvB], axis=1)
    wo = np.concatenate([woA, woB], axis=1)
    bias = b_out.reshape(4, 128, 1)

    in_maps = []
    for core in range(NCORES):
        b, half = core // 2, core % 2
        xs = np.ascontiguousarray(
            x[b, :, half * RPC:(half + 1) * RPC, :]).reshape(512, PIX)
        xh, xl = _pair(xs, SX)
        # layout [p, t, c, hl, j] -> [128, GRP*3072]
        xhr = xh.reshape(4, 128, GRP, GPIX)    # (c, p, t, j)
        xlr = xl.reshape(4, 128, GRP, GPIX)
        xA = np.ascontiguousarray(
            np.stack([xhr, xlr], axis=3).transpose(1, 2, 0, 3, 4)
        ).reshape(128, GRP * 3072)
        in_maps.append({"x": xA, "wqk": wqk, "wv": wv, "wo": wo,
                        "bias": bias})
    return in_maps


def unshard_outputs(results):
    out = np.empty((B, C, H, W), np.float32)
    for core in range(NCORES):
        b, half = core // 2, core % 2
        out[b, :, half * RPC:(half + 1) * RPC, :] = (
            np.asarray(results[core]["y"]).astype(np.float32)
            .reshape(C, RPC, W))
    return out


_NC_CACHE = None


def kernel(x, w_qkv, w_out, b_out):
    global _NC_CACHE
    from concourse.bass_utils import run_bass_kernel_spmd
    if _NC_CACHE is None:
        _NC_CACHE = build_nc()
    in_maps = shard_inputs(x, w_qkv, w_out, b_out)
    res = run_bass_kernel_spmd(_NC_CACHE, in_maps, list(range(NCORES)))
    return unshard_outputs(res.results)
